# revision 13
# baseline (speedup 1.0000x reference)
"""GAT 2-layer kernel for 8 Trainium2 NeuronCores — dma_gather edition.

Strategy (dst-node sharded, gather-based):
  - Nodes sharded 6250/core (padded to 6272 = 49*128). Edges (incl self
    loops) grouped by dst into per-core groups of 128 dst nodes, each
    group's edge list split into lo/hi halves by table row (< 32768 or
    not, since dma_gather indices are int16), each half padded to blocks
    of 128 edges.
  - Phase A: h = x @ W1_ext per shard (W1_ext cols: 96 h | asrc | adst)
    -> fp16 table shard [NSHP, 128] (cols 98+ zero); AllGather -> full
    table [50176, 128].
  - Edge phase per group g (128 dst nodes, nb = b_lo+b_hi blocks):
      * TWO dma_gather instructions fetch all the group's table rows
        (row i of the group's edge list -> partition i%128, block i//128)
      * per-edge dst logit via per-block one-hot transpose-mask matmuls
        (m10), z = pre + asrc, w = exp(leakyrelu(z) - 4)
      * matmul-accumulate PSUM[128 dst, 97] += m01_b.T @ [w*h | w];
        col 96 = softmax denominator
      * out1 = ELU(PSUM[:, :96]/denom + b1)
  - Layer2: h2 = out1 @ W2_ext -> compact [NSHP, 34] table, AllGather,
    expand into a 128-col padded local table, same edge phase with
    FL=32, then log_softmax -> out shard fp32.
"""
import sys
import time

sys.path.insert(0, "/opt/trn_rl_repo")

import numpy as np

N = 50000
E = 800000
F_IN = 256
HID = 96
N_CLS = 32
NEG_SLOPE = 0.2
N_CORES = 8
P = 128
NSH = 6250            # real nodes per core
NSHP = 6272           # padded (49 * 128)
NG = NSHP // P        # 49 groups per core
R1 = HID + 3          # W1_ext cols: h(96) | asrc | adst | zero = 99
R2 = N_CLS + 3        # W2_ext cols: h2(32) | asrc2 | adst2 | zero = 35
RP = 128              # padded table row (gather elem, 256B fp16)
R2C = N_CLS + 2       # compact table2 row: h2 | asrc2 | adst2 = 34
TAB = N_CORES * NSHP  # 50176
HALF = 32768          # int16 index range split
EXP_BIAS = -4.0

_CACHE = {}


def _wrap16(vals):
    """[n] int16 -> wrapped [16, n//16]: vals[i] at [i%16, i//16]."""
    n = len(vals)
    return np.ascontiguousarray(vals.reshape(n // 16, 16).T)


def _host_prep(edge_index):
    """Index preprocessing: per-core lo/hi split edge slots + masks."""
    import ml_dtypes

    src = np.asarray(edge_index[0], dtype=np.int64)
    dst = np.asarray(edge_index[1], dtype=np.int64)
    loops = np.arange(N, dtype=np.int64)
    src = np.concatenate([src, loops])
    dst = np.concatenate([dst, loops])
    order = np.argsort(dst, kind="stable")
    src = src[order]
    dst = dst[order]
    tab_row = (src // NSH) * NSHP + (src % NSH)

    # pass 1: per-core per-group lo/hi segments
    segs = []  # [core][group] -> (lo_idx, lo_rel, hi_idx, hi_rel)
    for c in range(N_CORES):
        a, b = np.searchsorted(dst, [c * NSH, (c + 1) * NSH])
        s_c = tab_row[a:b]
        d_c = dst[a:b] - c * NSH
        g_c = d_c // P
        rows = []
        for g in range(NG):
            m = g_c == g
            sg = s_c[m]
            rg = (d_c[m] - g * P).astype(np.int64)
            present = np.zeros(P, dtype=bool)
            present[rg] = True
            missing = np.nonzero(~present)[0]
            if len(missing):
                # pad nodes with no edges: point at row 0 so denom > 0
                sg = np.concatenate([sg, np.zeros(len(missing), np.int64)])
                rg = np.concatenate([rg, missing])
            lo = sg < HALF
            rows.append((sg[lo], rg[lo], sg[~lo] - HALF, rg[~lo]))
        segs.append(rows)

    b_lo = np.zeros(NG, np.int64)
    b_hi = np.zeros(NG, np.int64)
    for g in range(NG):
        for c in range(N_CORES):
            lo_i, _, hi_i, _ = segs[c][g]
            b_lo[g] = max(b_lo[g], -(-len(lo_i) // P))
            b_hi[g] = max(b_hi[g], -(-len(hi_i) // P))
    nb = b_lo + b_hi
    nbt = int(nb.sum())

    cores = []
    for c in range(N_CORES):
        idx16 = np.zeros((16, nbt * 8), np.int16)
        rel_all = np.full((nbt, P), -1, np.int16)
        boff = 0
        woff = 0
        for g in range(NG):
            lo_i, lo_r, hi_i, hi_r = segs[c][g]
            for seg_i, seg_r, nblk in ((lo_i, lo_r, int(b_lo[g])),
                                       (hi_i, hi_r, int(b_hi[g]))):
                if nblk == 0:
                    continue
                n = nblk * P
                ii = np.zeros(n, np.int16)
                ii[: len(seg_i)] = seg_i.astype(np.int16)
                rr = np.full(n, -1, np.int16)
                rr[: len(seg_r)] = seg_r.astype(np.int16)
                idx16[:, woff:woff + nblk * 8] = _wrap16(ii)
                rel_all[boff:boff + nblk] = rr.reshape(nblk, P)
                boff += nblk
                woff += nblk * 8
        # one-hot masks fp8: m01[e, b*128+c] = 1 iff edge(b,e) has dst col c
        cols = np.arange(P, dtype=np.int16)
        oh = (rel_all[:, :, None] == cols[None, None, :]).astype(
            ml_dtypes.float8_e4m3)
        m01 = np.ascontiguousarray(
            np.transpose(oh, (1, 0, 2)).reshape(P, nbt * P))
        m10 = np.ascontiguousarray(
            np.transpose(oh, (2, 0, 1)).reshape(P, nbt * P))
        cores.append(dict(idx16=np.tile(idx16, (8, 1)), m01=m01, m10=m10))
    return b_lo, b_hi, cores


def _split_multi_waits(nc):
    """TRN2 ISA has one sync-wait slot per instruction; Tile sometimes emits
    more. Hoist extras onto preceding single-wait Drain pseudo-instructions."""
    import concourse.mybir as mybir

    for f in nc.m.functions:
        for blk in f.blocks:
            out = []
            for inst in blk.instructions:
                si = inst.sync_info
                if si is not None and len(si.on_wait) > 1:
                    waits = list(si.on_wait)
                    for w in waits[:-1]:
                        d = mybir.InstNoOp(
                            name=nc.get_next_instruction_name(),
                            ins=[], outs=[],
                        )
                        d.engine = inst.engine
                        d.sync_info = mybir.SyncInfo(on_wait=[w], on_update=[])
                        out.append(d)
                    inst.sync_info = mybir.SyncInfo(
                        on_wait=[waits[-1]], on_update=list(si.on_update))
                out.append(inst)
            blk.instructions = out


class _SpmdRunner:
    """Persistent jitted 8-core runner (mirrors bass2jax.run_bass_via_pjrt)."""

    def __init__(self, nc, n_cores=8):
        import jax
        from jax.sharding import Mesh, PartitionSpec, NamedSharding
        from jax.experimental.shard_map import shard_map
        import concourse.mybir as mybir
        from concourse.bass2jax import (
            _bass_exec_p, install_neuronx_cc_hook, partition_id_tensor)

        install_neuronx_cc_hook()
        self.jax = jax
        self.n_cores = n_cores
        pname = nc.partition_id_tensor.name if nc.partition_id_tensor else None
        in_names, out_names, out_avals, zero_outs = [], [], [], []
        for alloc in nc.m.functions[0].allocations:
            if not isinstance(alloc, mybir.MemoryLocationSet):
                continue
            name = alloc.memorylocations[0].name
            if alloc.kind == "ExternalInput":
                if name != pname:
                    in_names.append(name)
            elif alloc.kind == "ExternalOutput":
                shape = tuple(alloc.tensor_shape)
                dtype = mybir.dt.np(alloc.dtype)
                out_names.append(name)
                out_avals.append(jax.core.ShapedArray(shape, dtype))
                zero_outs.append(np.zeros(shape, dtype))
        self.in_names, self.out_names = in_names, out_names
        self.out_avals, self.zero_outs = out_avals, zero_outs
        n_params, n_outs = len(in_names), len(out_avals)
        all_in = in_names + out_names + ([pname] if pname else [])

        def _body(*args):
            operands = list(args)
            if pname is not None:
                operands.append(partition_id_tensor())
            return tuple(_bass_exec_p.bind(
                *operands, out_avals=tuple(out_avals), in_names=tuple(all_in),
                out_names=tuple(out_names), lowering_input_output_aliases=(),
                sim_require_finite=True, sim_require_nnan=True, nc=nc))

        devices = jax.devices()[:n_cores]
        mesh = Mesh(np.asarray(devices), ("core",))
        # Staging MUST use the mesh sharding: an unsharded device_put parks
        # the arrays on one device and every execution re-scatters them.
        self.sharding = NamedSharding(mesh, PartitionSpec("core"))
        self.fn = jax.jit(
            shard_map(_body, mesh=mesh,
                      in_specs=(PartitionSpec("core"),) * (n_params + n_outs),
                      out_specs=(PartitionSpec("core"),) * len(out_names),
                      check_rep=False),
            donate_argnums=tuple(range(n_params, n_params + n_outs)),
            keep_unused=True)
        self.n_params = n_params
        self.staged = None

    def stage(self, in_maps):
        jax = self.jax
        per_core = [[np.asarray(m[n]) for n in self.in_names] for m in in_maps]
        self.staged = jax.device_put([
            np.concatenate([per_core[c][i] for c in range(self.n_cores)], axis=0)
            for i in range(self.n_params)], self.sharding)
        jax.block_until_ready(self.staged)

    def _zeros(self):
        return [np.zeros((self.n_cores * z.shape[0], *z.shape[1:]), z.dtype)
                for z in self.zero_outs]

    def run_results(self):
        jax = self.jax
        z = jax.device_put(self._zeros(), self.sharding)
        out = self.fn(*self.staged, *z)
        jax.block_until_ready(out)
        return [
            {n: np.asarray(out[i]).reshape(self.n_cores, *self.out_avals[i].shape)[c]
             for i, n in enumerate(self.out_names)}
            for c in range(self.n_cores)]

    def time_min(self, iters=6, warmup=2):
        jax = self.jax
        for _ in range(warmup):
            z = jax.device_put(self._zeros(), self.sharding)
            jax.block_until_ready(self.fn(*self.staged, *z))
        ts = []
        for _ in range(iters):
            z = jax.device_put(self._zeros(), self.sharding)
            jax.block_until_ready(z)
            t0 = time.perf_counter()
            jax.block_until_ready(self.fn(*self.staged, *z))
            ts.append(time.perf_counter() - t0)
        return min(ts)


def _build_nc(b_lo, b_hi, ablate=()):
    import concourse.bass as bass
    import concourse.mybir as mybir
    import concourse.tile as tile
    from concourse import library_config
    from concourse.library_overlay import lower_extended_insts

    fp16 = mybir.dt.float16
    fp32 = mybir.dt.float32
    fp8 = mybir.dt.float8e4
    AO = mybir.AluOpType
    AF = mybir.ActivationFunctionType

    nb = [int(b_lo[g] + b_hi[g]) for g in range(NG)]
    nbt = int(sum(nb))

    nc = bass.Bass()
    xT = nc.declare_dram_parameter("xT", [F_IN, NSHP], fp16, isOutput=False)
    w1 = nc.declare_dram_parameter("w1", [F_IN, R1], fp16, isOutput=False)
    w2 = nc.declare_dram_parameter("w2", [HID, R2], fp16, isOutput=False)
    b1r = nc.declare_dram_parameter("b1r", [1, HID], fp16, isOutput=False)
    b2r = nc.declare_dram_parameter("b2r", [1, N_CLS], fp16, isOutput=False)
    iden = nc.declare_dram_parameter("iden", [P, P], fp16, isOutput=False)
    idx16 = nc.declare_dram_parameter(
        "idx16", [P, nbt * 8], mybir.dt.int16, isOutput=False)
    m01 = nc.declare_dram_parameter(
        "m01", [P, nbt * P], mybir.dt.float8e4, isOutput=False)
    m10 = nc.declare_dram_parameter(
        "m10", [P, nbt * P], mybir.dt.float8e4, isOutput=False)
    out = nc.declare_dram_parameter("out", [NSH, N_CLS], fp32, isOutput=True)

    nc.gpsimd.load_library(library_config.mlp)

    with tile.TileContext(nc) as tc:
        with (
            tc.tile_pool(name="const", bufs=1) as cp,
            tc.tile_pool(name="sb", bufs=3) as sb,
            tc.tile_pool(name="ps", bufs=3, space="PSUM") as ps,
            tc.tile_pool(name="pt", bufs=2, space="PSUM") as pt,
            tc.tile_pool(name="dram", bufs=1, space="DRAM") as dr,
        ):
            ident = cp.tile([P, P], fp16)
            nc.sync.dma_start(out=ident[:], in_=iden[:])
            w1t = cp.tile([P, 2, R1], fp16)
            nc.sync.dma_start(out=w1t[:], in_=w1[:].rearrange("(k p) r -> p k r", p=P))
            w2t = cp.tile([HID, R2], fp16)
            nc.sync.dma_start(out=w2t[:], in_=w2[:])
            idx_sb = cp.tile([P, nbt * 8], mybir.dt.int16)
            nc.sync.dma_start(out=idx_sb[:], in_=idx16[:])

            # b replicated tiles via transpose trick
            ones1 = cp.tile([1, P], fp16)
            nc.vector.memset(ones1[:], 1.0)
            b1h = cp.tile([1, HID], fp16)
            nc.sync.dma_start(out=b1h[:], in_=b1r[:])
            b2h = cp.tile([1, N_CLS], fp16)
            nc.sync.dma_start(out=b2h[:], in_=b2r[:])
            b1ps = pt.tile([P, HID], fp32, space="PSUM", tag="tp")
            nc.tensor.matmul(out=b1ps[:], lhsT=ones1[:], rhs=b1h[:], start=True, stop=True)
            b1rep = cp.tile([P, HID], fp32)
            nc.vector.tensor_copy(b1rep[:], b1ps[:])
            b2ps = pt.tile([P, N_CLS], fp32, space="PSUM", tag="tp")
            nc.tensor.matmul(out=b2ps[:], lhsT=ones1[:], rhs=b2h[:], start=True, stop=True)
            b2rep = cp.tile([P, N_CLS], fp32)
            nc.vector.tensor_copy(b2rep[:], b2ps[:])
            neg4 = cp.tile([P, 1], fp32)
            nc.vector.memset(neg4[:], EXP_BIAS)

            # pre-allocate num_idxs registers (one per distinct value) —
            # Tile disables expression caching, so per-call to_reg would
            # exhaust the Pool register file after ~48 gathers
            nvals = sorted({int(b_lo[g]) * P for g in range(NG)} |
                           {int(b_hi[g]) * P for g in range(NG)})
            nreg = {v: nc.gpsimd.to_reg(v) for v in nvals if v}

            tab1_sh = dr.tile([NSHP, RP], fp16)
            tab1_cc = dr.tile([TAB, RP], fp16, addr_space="Shared")
            tab1 = dr.tile([TAB, RP], fp16)
            tab2_sh = dr.tile([NSHP, R2C], fp16)
            tab2_cc = dr.tile([TAB, R2C], fp16, addr_space="Shared")
            tab2 = dr.tile([TAB, RP], fp16)

            # ---- Phase A: h table shard ----
            xTc = cp.tile([P, 2, NSHP], fp16)
            nc.sync.dma_start(out=xTc[:], in_=xT[:].rearrange("(k p) n -> p k n", p=P))
            for g in range(NG):
                hps = ps.tile([P, R1], fp32, space="PSUM", tag="agg")
                for k in range(2):
                    nc.tensor.matmul(
                        out=hps[:], lhsT=xTc[:, k, g * P:(g + 1) * P],
                        rhs=w1t[:, k, :], start=(k == 0), stop=(k == 1),
                    )
                hsb = sb.tile([P, RP], fp16, tag="hsb")
                nc.scalar.activation(out=hsb[:, 0:R1 - 1], in_=hps[:, 0:R1 - 1],
                                     func=AF.Copy, bias=0.0, scale=1.0)
                nc.vector.memset(hsb[:, R1 - 1:RP], 0.0)
                nc.sync.dma_start(out=tab1_sh[g * P:(g + 1) * P, :], in_=hsb[:])
            if "nocollective" in ablate:
                nc.sync.dma_start(out=tab1_cc[0:NSHP, :], in_=tab1_sh[:])
            else:
                nc.gpsimd.collective_compute(
                    "AllGather", mybir.AluOpType.bypass,
                    replica_groups=[list(range(N_CORES))],
                    ins=[tab1_sh.opt()], outs=[tab1_cc.opt()],
                )
            nc.sync.dma_start(out=tab1[:], in_=tab1_cc[:])

            def run_layer(table_sh, table, RLr, FL, brep, tag, post):
                boff = 0
                for g in range(NG):
                    nbg = nb[g]
                    gt = sb.tile([P, nbg, RP], fp16, tag=f"gt{tag}", bufs=6)
                    if "nogather" in ablate:
                        nc.sync.dma_start(
                            out=gt[:], in_=table[0:P * nbg, :].rearrange(
                                "(b p) r -> p b r", p=P))
                    else:
                        blo, bhi = int(b_lo[g]), int(b_hi[g])
                        if blo:
                            nc.gpsimd.dma_gather(
                                gt[:, 0:blo, :], table[0:HALF, :],
                                idx_sb[:, boff * 8:(boff + blo) * 8],
                                blo * P, nreg[blo * P], RP,
                                single_packet=False)
                        if bhi:
                            nc.gpsimd.dma_gather(
                                gt[:, blo:nbg, :], table[HALF:TAB, :],
                                idx_sb[:, (boff + blo) * 8:(boff + nbg) * 8],
                                bhi * P, nreg[bhi * P], RP,
                                single_packet=False)
                    adst = sb.tile([P, 1], fp16, tag=f"ad{tag}")
                    nc.sync.dma_start(
                        out=adst[:],
                        in_=table_sh[g * P:(g + 1) * P, RLr - 2:RLr - 1],
                    )
                    m01s = sb.tile([P, nbg, P], fp8, tag=f"m01{tag}")
                    nc.sync.dma_start(
                        out=m01s[:],
                        in_=m01[:, boff * P:(boff + nbg) * P].rearrange(
                            "p (b c) -> p b c", c=P),
                    )
                    m10s = sb.tile([P, nbg, P], fp8, tag=f"m10{tag}")
                    nc.sync.dma_start(
                        out=m10s[:],
                        in_=m10[:, boff * P:(boff + nbg) * P].rearrange(
                            "p (b c) -> p b c", c=P),
                    )
                    # per-edge dst logit via one-hot transpose mask matmuls
                    pre = ps.tile([P, nbg, 1], fp32, space="PSUM", tag="pre")
                    for b in range(nbg):
                        nc.tensor.matmul(
                            out=pre[:, b, :], lhsT=m10s[:, b, :], rhs=adst[:],
                            start=True, stop=True,
                        )
                    # w = exp(leakyrelu(pre + asrc) - 4)  [P, nbg, 1]
                    z = sb.tile([P, nbg, 1], fp32, tag=f"z{tag}")
                    nc.vector.tensor_tensor(
                        out=z[:], in0=pre[:], in1=gt[:, :, FL:FL + 1],
                        op=AO.add,
                    )
                    e02 = sb.tile([P, nbg, 1], fp32, tag=f"e02{tag}")
                    nc.vector.tensor_scalar(
                        out=e02[:], in0=z[:], scalar1=NEG_SLOPE,
                        scalar2=None, op0=AO.mult,
                    )
                    nc.vector.tensor_tensor(out=z[:], in0=z[:], in1=e02[:],
                                            op=AO.max)
                    w = sb.tile([P, nbg, 1], fp16, tag=f"w{tag}")
                    nc.scalar.activation(out=w[:], in_=z[:],
                                         func=AF.Exp, bias=neg4[:], scale=1.0)
                    # weighted rows: gt2 = [w*h | w]
                    gt2 = sb.tile([P, nbg, FL + 1], fp16, tag=f"gt2{tag}")
                    nc.vector.tensor_tensor(
                        out=gt2[:, :, 0:FL], in0=gt[:, :, 0:FL],
                        in1=w[:].to_broadcast([P, nbg, FL]), op=AO.mult,
                    )
                    nc.vector.tensor_copy(gt2[:, :, FL:FL + 1], w[:])
                    # aggregate: agg[d, :] = sum_e onehot[e,d] * gt2[e, :]
                    agg = ps.tile([P, FL + 1], fp32, space="PSUM", tag="agg")
                    for b in range(nbg):
                        nc.tensor.matmul(
                            out=agg[:], lhsT=m01s[:, b, :], rhs=gt2[:, b, :],
                            start=(b == 0), stop=(b == nbg - 1),
                        )
                    # divide + bias
                    rcp = sb.tile([P, 1], fp32, tag=f"rcp{tag}")
                    nc.vector.reciprocal(rcp[:], agg[:, FL:FL + 1])
                    o = sb.tile([P, FL], fp32, tag=f"o{tag}")
                    nc.vector.tensor_scalar(
                        out=o[:], in0=agg[:, 0:FL], scalar1=rcp[:],
                        scalar2=None, op0=AO.mult,
                    )
                    nc.vector.tensor_tensor(out=o[:], in0=o[:], in1=brep[:],
                                            op=AO.add)
                    post(g, o)
                    boff += nbg

            def post1(g, o):
                # elu = max(u,0) + exp(min(u,0)) - 1
                mn = sb.tile([P, HID], fp32, tag="mn")
                nc.vector.tensor_scalar(out=mn[:], in0=o[:], scalar1=0.0,
                                        scalar2=None, op0=AO.min)
                ex = sb.tile([P, HID], fp32, tag="ex")
                nc.scalar.activation(out=ex[:], in_=mn[:], func=AF.Exp,
                                     bias=0.0, scale=1.0)
                mx = sb.tile([P, HID], fp16, tag="mx")
                nc.vector.tensor_scalar(out=mx[:], in0=o[:], scalar1=0.0,
                                        scalar2=-1.0, op0=AO.max, op1=AO.add)
                elu = sb.tile([P, HID], fp16, tag="elu")
                nc.vector.tensor_tensor(out=elu[:], in0=ex[:], in1=mx[:],
                                        op=AO.add)
                # transpose [128, 96] -> [96, 128]
                elups = pt.tile([HID, P], fp16, space="PSUM", tag="tp")
                nc.tensor.transpose(out=elups[:], in_=elu[:], identity=ident[:])
                eluT = sb.tile([HID, P], fp16, tag="eluT")
                nc.vector.tensor_copy(eluT[:], elups[:])
                h2ps = ps.tile([P, R2], fp32, space="PSUM", tag="agg")
                nc.tensor.matmul(out=h2ps[:], lhsT=eluT[:], rhs=w2t[:],
                                 start=True, stop=True)
                h2sb = sb.tile([P, R2C], fp16, tag="h2sb")
                nc.scalar.activation(out=h2sb[:], in_=h2ps[:, 0:R2C],
                                     func=AF.Copy, bias=0.0, scale=1.0)
                nc.sync.dma_start(out=tab2_sh[g * P:(g + 1) * P, :], in_=h2sb[:])

            def post2(g, o):
                if g * P >= NSH:
                    return
                mx2 = sb.tile([P, 1], fp32, tag="mx2")
                nc.vector.tensor_reduce(
                    out=mx2[:], in_=o[:], op=AO.max,
                    axis=mybir.AxisListType.X,
                )
                t = sb.tile([P, N_CLS], fp32, tag="t2")
                nc.vector.tensor_scalar(out=t[:], in0=o[:], scalar1=mx2[:],
                                        scalar2=None, op0=AO.subtract)
                exs = sb.tile([P, N_CLS], fp32, tag="exs")
                ssum = sb.tile([P, 1], fp32, tag="ssum")
                nc.scalar.activation(out=exs[:], in_=t[:], func=AF.Exp,
                                     bias=0.0, scale=1.0, accum_out=ssum[:])
                lse = sb.tile([P, 1], fp32, tag="lse")
                nc.scalar.activation(out=lse[:], in_=ssum[:], func=AF.Ln,
                                     bias=0.0, scale=1.0)
                fo = sb.tile([P, N_CLS], fp32, tag="fo")
                nc.vector.tensor_scalar(out=fo[:], in0=t[:], scalar1=lse[:],
                                        scalar2=None, op0=AO.subtract)
                hi = min((g + 1) * P, NSH)
                nc.sync.dma_start(out=out[g * P:hi, :], in_=fo[: hi - g * P, :])

            run_layer(tab1_sh, tab1, R1, HID, b1rep, "L1", post1)
            if "nocollective" in ablate:
                nc.sync.dma_start(out=tab2_cc[0:NSHP, :], in_=tab2_sh[:])
            else:
                nc.gpsimd.collective_compute(
                    "AllGather", mybir.AluOpType.bypass,
                    replica_groups=[list(range(N_CORES))],
                    ins=[tab2_sh.opt()], outs=[tab2_cc.opt()],
                )
            # expand compact table2 into the 128-col padded gather layout
            nc.sync.dma_start(out=tab2[:, 0:R2C], in_=tab2_cc[:])
            run_layer(tab2_sh, tab2, R2, N_CLS, b2rep, "L2", post2)

    _split_multi_waits(nc)
    lower_extended_insts(nc)
    return nc


def kernel(x, edge_index, W1, a1_src, a1_dst, b1, W2, a2_src, a2_dst, b2):
    x = np.asarray(x, np.float32)
    if "prep" not in _CACHE:
        _CACHE["prep"] = _host_prep(edge_index)
    b_lo, b_hi, cores = _CACHE["prep"]

    W1e = np.concatenate(
        [W1, (W1 @ a1_src)[:, None], (W1 @ a1_dst)[:, None],
         np.zeros((F_IN, 1), np.float32)], axis=1).astype(np.float16)
    W2e = np.concatenate(
        [W2, (W2 @ a2_src)[:, None], (W2 @ a2_dst)[:, None],
         np.zeros((HID, 1), np.float32)], axis=1).astype(np.float16)
    iden = np.eye(P, dtype=np.float16)
    in_maps = []
    for c in range(N_CORES):
        d = cores[c]
        xs = x[c * NSH:(c + 1) * NSH]
        xT = np.zeros((F_IN, NSHP), np.float16)
        xT[:, :NSH] = xs.T.astype(np.float16)
        in_maps.append({
            "xT": xT, "w1": W1e, "w2": W2e,
            "b1r": np.asarray(b1, np.float16)[None, :],
            "b2r": np.asarray(b2, np.float16)[None, :],
            "iden": iden, "idx16": d["idx16"],
            "m01": d["m01"], "m10": d["m10"],
        })

    if "runner" not in _CACHE:
        nc = _build_nc(b_lo, b_hi)
        _CACHE["runner"] = _SpmdRunner(nc, N_CORES)
    run = _CACHE["runner"]
    run.stage(in_maps)
    res = run.run_results()
    return np.concatenate([res[c]["out"] for c in range(N_CORES)], axis=0)


def measure_hw_ns(iters=6):
    """Steady-state wall time of the staged kernel minus a no-op dispatch
    baseline of the same I/O shape class (axon per-execution overhead)."""
    run = _CACHE.get("runner")
    assert run is not None and run.staged is not None, "call kernel() first"
    if "noop" not in _CACHE:
        import concourse.bass as bass
        import concourse.mybir as mybir
        import concourse.tile as tile
        nc = bass.Bass()
        a = nc.declare_dram_parameter("a", [128, 32], mybir.dt.float32, isOutput=False)
        o = nc.declare_dram_parameter("out", [128, 32], mybir.dt.float32, isOutput=True)
        with tile.TileContext(nc) as tc:
            with tc.tile_pool(name="s", bufs=1) as sb:
                t = sb.tile([128, 32], mybir.dt.float32)
                nc.sync.dma_start(out=t[:], in_=a[:])
                nc.sync.dma_start(out=o[:], in_=t[:])
        _split_multi_waits(nc)
        nr = _SpmdRunner(nc, N_CORES)
        nr.stage([{"a": np.zeros((128, 32), np.float32)}] * N_CORES)
        _CACHE["noop"] = nr
    nr = _CACHE["noop"]
    # interleave to cancel slow drift in axon dispatch overhead
    reals, bases = [], []
    for _ in range(max(iters, 14)):
        bases.append(nr.time_min(iters=1, warmup=0))
        reals.append(run.time_min(iters=1, warmup=0))
    return max(int((min(reals) - min(bases)) * 1e9), 1000)


def _to_fp8(a):
    import ml_dtypes
    return a.astype(ml_dtypes.float8_e4m3)
